# revision 47
# baseline (speedup 1.0000x reference)
"""Multi-head causal attention (SEQ=4096, D=1024, H=16, DK=64) on 8 TRN2
NeuronCores, tensor-parallel over heads (2 heads/core). Self-contained.

All matmuls single-product f32r (~13-bit products; measured rel err ~9e-3
vs the 2e-2 gate). Per-core pipeline, emitted as interleaved instruction
streams so PE/ACT/DVE/DMA stay co-busy:

  per 512-query block nb:
    - filler pieces (generators, drained between attention tiles):
      K/Q/V projections for block nb+1 (W.T @ X^T, f32r, DMA-streamed),
      stats pass for nb+1 (S tiles + DVE row-max -> -m rides as the 65th
      contraction row of Qh^T), W_O for block nb-1.
    - main: both heads' S^T pass interleaved at kc granularity:
      S^T[kc,q] = [Kh;1].T @ [Qh;-m] (one f32r matmul + diag mask via
      identity-matmul), ACT exp -> P^T, AV with ones-augmented Vh
      (l = sum(exp) rides as PSUM row 64).
    - epilogue: C^T/l drained; r = 1/l broadcast to R[dk,q] via a rank-2
      PE matmul; C^T *= R on DVE. W_O then needs ONE matmul per output
      tile (contraction 128 = both heads), y emitted bf16 (host sums
      partials in fp32).
"""

import os
import sys
from collections import deque

sys.path.insert(0, "/opt/trn_rl_repo")

import numpy as np
import ml_dtypes

import concourse.bass as bass
import concourse.mybir as mybir
import concourse.tile as tile
from concourse.bass_utils import run_bass_kernel_spmd
from concourse.masks import make_identity

P = 128
S = 4096
D = 1024
DK = 64
NH = 2  # heads per core
NCORES = 8
NEG = -1.0e9
F32 = mybir.dt.float32
F32R = mybir.dt.float32r
BF16 = mybir.dt.bfloat16
EXP = mybir.ActivationFunctionType.Exp

_ctr = [0]


def _split_waits(nc, max_waits=1):
    """walrus rejects >1 sem-wait per instruction; move extras onto
    preceding same-engine NOPs (engine streams are program-ordered)."""
    for f in nc.m.functions:
        for bb in f.blocks:
            insts = bb.instructions
            new = []
            changed = False
            for inst in insts:
                si = inst.sync_info
                if si is not None and si.on_wait and len(si.on_wait) > max_waits:
                    waits = list(si.on_wait)
                    extra, keep = waits[:-max_waits], waits[-max_waits:]
                    for i in range(0, len(extra), max_waits):
                        _ctr[0] += 1
                        new.append(
                            mybir.InstNoOp(
                                name=f"waitsplit-{_ctr[0]}",
                                engine=inst.engine,
                                ins=[],
                                outs=[],
                                sync_info=mybir.SyncInfo(
                                    on_wait=extra[i : i + max_waits], on_update=[]
                                ),
                            )
                        )
                    inst.sync_info = mybir.SyncInfo(
                        on_wait=keep, on_update=list(si.on_update)
                    )
                    changed = True
                new.append(inst)
            if changed:
                bb.instructions = new


def build(nc: bass.Bass, causal: bool = True):
    repeat = int(os.environ.get("ATTN_REPEAT", "1"))
    NB = S // 512  # 8   512-wide blocks
    QB = S // P  # 32  128-wide q blocks
    DC = D // P  # 8   128-deep contraction chunks

    qT = nc.dram_tensor("qT", [D, S], F32R, kind="ExternalInput")
    kT = nc.dram_tensor("kT", [D, S], F32R, kind="ExternalInput")
    vT = nc.dram_tensor("vT", [D, S], F32R, kind="ExternalInput")
    # weights pre-transposed host-side to [P, DC*P] so the load is contiguous
    wq = nc.dram_tensor("wq", [P, DC * P], F32R, kind="ExternalInput")
    wk = nc.dram_tensor("wk", [P, DC * P], F32R, kind="ExternalInput")
    wv = nc.dram_tensor("wv", [P, DC * P], F32R, kind="ExternalInput")
    wo = nc.dram_tensor("wo", [NH * DK, D], F32R, kind="ExternalInput")
    maskf = nc.dram_tensor("maskf", [P, P], BF16, kind="ExternalInput")
    maskb = nc.dram_tensor("maskb", [P, P], BF16, kind="ExternalInput")
    onesr = nc.dram_tensor("onesr", [1, S], F32R, kind="ExternalInput")
    selr = nc.dram_tensor("selr", [2, P], F32R, kind="ExternalInput")
    y0 = nc.dram_tensor("y0", [S, D], BF16, kind="ExternalOutput")

    with tile.TileContext(nc) as tc:
        import contextlib

        ctx = contextlib.ExitStack()
        with ctx:
            const = ctx.enter_context(tc.tile_pool(name="const", bufs=1))
            big = ctx.enter_context(tc.tile_pool(name="big", bufs=1))
            stream = ctx.enter_context(
                tc.tile_pool(name="stream", bufs=int(os.environ.get("ATTN_BSTREAM", "28")))
            )
            ptp = ctx.enter_context(
                tc.tile_pool(name="ptp", bufs=int(os.environ.get("ATTN_BPT", "6")))
            )
            ypool = ctx.enter_context(
                tc.tile_pool(name="ypool", bufs=int(os.environ.get("ATTN_BY", "3")))
            )
            smalls = ctx.enter_context(tc.tile_pool(name="smalls", bufs=2))
            bproj = int(os.environ.get("ATTN_BPROJ", "2"))
            bstat = int(os.environ.get("ATTN_BSTAT", "2"))
            bst = int(os.environ.get("ATTN_BST", "2"))
            bmisc = int(os.environ.get("ATTN_BMISC", "2"))
            ps_proj = ctx.enter_context(
                tc.tile_pool(name="ps_proj", bufs=bproj, space="PSUM")
            )
            ps_stat = ctx.enter_context(
                tc.tile_pool(name="ps_stat", bufs=bstat, space="PSUM")
            )
            ps_st = ctx.enter_context(tc.tile_pool(name="ps_st", bufs=bst, space="PSUM"))
            ps_misc = ctx.enter_context(
                tc.tile_pool(name="ps_misc", bufs=bmisc, space="PSUM")
            )

            # ---- constants ----
            ident = const.tile([P, P], F32)
            make_identity(nc, ident[:])
            ident_b = const.tile([P, P], BF16)
            nc.vector.tensor_copy(ident_b[:], ident[:])
            ident_r = const.tile([P, P], F32R)
            nc.vector.tensor_copy(ident_r[:], ident[:])

            # K first: the first compute is the K-projection of block 0
            wq_sb = const.tile([P, DC, P], F32R, tag="wq")
            wk_sb = const.tile([P, DC, P], F32R, tag="wk")
            wv_sb = const.tile([P, DC, P], F32R, tag="wv")
            nc.sync.dma_start(wk_sb[:], wk.rearrange("p (o m) -> p o m", o=DC))
            nc.sync.dma_start(wq_sb[:], wq.rearrange("p (o m) -> p o m", o=DC))
            nc.sync.dma_start(wv_sb[:], wv.rearrange("p (o m) -> p o m", o=DC))
            wo_sb = const.tile([P, D], F32R, tag="wo")
            nc.sync.dma_start(wo_sb[:], wo[:])

            mf_sb = const.tile([P, P], BF16, tag="mf")
            mb_sb = const.tile([P, P], BF16, tag="mb")
            nc.sync.dma_start(mf_sb[:], maskf[:])
            nc.sync.dma_start(mb_sb[:], maskb[:])

            # head-half selector for the rank-2 broadcast of 1/l
            sel = const.tile([2, P], F32R, tag="sel")
            nc.sync.dma_start(sel[:], selr[:])

            # ---- persistent activations ----
            qhT = [big.tile([P, S], F32R, tag=f"qhT{h}", name=f"qhT{h}") for h in range(NH)]
            khT = [big.tile([P, S], F32R, tag=f"khT{h}", name=f"khT{h}") for h in range(NH)]
            vh = [big.tile([P, QB, DK + 1], F32R, tag=f"vh{h}", name=f"vh{h}") for h in range(NH)]
            ct = big.tile([P, S], F32R, tag="ct")
            mcol = [big.tile([P, QB], F32, tag=f"mcol{h}", name=f"mcol{h}") for h in range(NH)]
            mcol_r = [big.tile([P, QB], F32R, tag=f"mcolr{h}", name=f"mcolr{h}") for h in range(NH)]
            # l rows live in dead partitions of qhT[h] (row 96)
            lcol = [big.tile([P, QB], F32R, tag=f"lcol{h}", name=f"lcol{h}") for h in range(NH)]
            rcol = [big.tile([P, QB], F32, tag=f"rcol{h}", name=f"rcol{h}") for h in range(NH)]
            rcol_r = [big.tile([P, QB], F32R, tag=f"rcolr{h}", name=f"rcolr{h}") for h in range(NH)]
            rrow = big.tile([2, S], F32R, tag="rrow")

            ones_qb = const.tile([P, QB], F32, tag="ones_qb")
            nc.any.memset(ones_qb[:], 1.0)
            for h in range(NH):
                nc.sync.dma_start(khT[h][DK : DK + 1, :], onesr[:])  # ones row
                nc.vector.tensor_copy(vh[h][:, :, DK], ones_qb[:])  # ones col

            for _rep in range(repeat):
                # ------- filler generators (emit a chunk per next()) -------

                def gen_qkproj(t_idx, nb):
                    xdram, w_sb = [(qT, wq_sb), (kT, wk_sb)][t_idx]
                    xts = []
                    for dc in range(DC):
                        xt = stream.tile([P, 512], F32R, tag="xin", name="xt")
                        nc.sync.dma_start(
                            xt[:],
                            xdram[dc * P : (dc + 1) * P, nb * 512 : (nb + 1) * 512],
                        )
                        xts.append(xt)
                    yield
                    ps = ps_proj.tile([P, 512], F32, tag="proj", name="ps")
                    for dc in range(DC):
                        nc.tensor.matmul(
                            ps[:],
                            w_sb[:, dc, :],
                            xts[dc][:],
                            start=(dc == 0),
                            stop=(dc == DC - 1),
                        )
                        if dc % 4 == 3:
                            yield
                    dst = qhT if t_idx == 0 else khT
                    for h in range(NH):
                        sl = slice(nb * 512, (nb + 1) * 512)
                        nc.scalar.copy(dst[h][0:DK, sl], ps[h * DK : (h + 1) * DK, :])
                    yield

                def gen_vproj(nb):
                    xts = []
                    for dc in range(DC):
                        xt = stream.tile([P, 512], F32R, tag="xin", name="xtv")
                        nc.sync.dma_start(
                            xt[:],
                            vT[dc * P : (dc + 1) * P, nb * 512 : (nb + 1) * 512],
                        )
                        xts.append(xt)
                    yield
                    ps = ps_proj.tile([P, 512], F32, tag="proj", name="ps")
                    for dc in range(DC):
                        nc.tensor.matmul(
                            ps[:],
                            wv_sb[:, dc, :],
                            xts[dc][:],
                            start=(dc == 0),
                            stop=(dc == DC - 1),
                        )
                        if dc % 4 == 3:
                            yield
                    vtmp = stream.tile([P, 512], F32R, tag="xin", name="vtmp")
                    nc.vector.tensor_copy(vtmp[:], ps[:])
                    yield
                    for h in range(NH):
                        pst = ps_stat.tile([P, 512], F32R, tag="stat", name="pst")
                        for j in range(4):
                            nc.tensor.transpose(
                                pst[0:P, j * DK : (j + 1) * DK],
                                vtmp[h * DK : (h + 1) * DK, j * P : (j + 1) * P],
                                ident_r[h * DK : (h + 1) * DK, h * DK : (h + 1) * DK],
                            )
                        nc.vector.tensor_copy(
                            vh[h][:, nb * 4 : nb * 4 + 4, 0:DK],
                            pst[:, 0 : 4 * DK].rearrange("p (j d) -> p j d", j=4),
                        )
                        yield

                def stats_mm(h, qb, kc):
                    ps = ps_stat.tile([P, 512], F32, tag="stat", name="ps_stat")
                    diag = causal and (kc == qb // 4)
                    nv = (qb % 4) * P + P if diag else 512
                    nc.tensor.matmul(
                        ps[:, 0:nv],
                        qhT[h][0:DK, qb * P : (qb + 1) * P],
                        khT[h][0:DK, kc * 512 : kc * 512 + nv],
                        start=True,
                        stop=not diag,
                    )
                    if diag:
                        nc.tensor.matmul(
                            ps[:, nv - P : nv],
                            ident_b[:],
                            mb_sb[:],
                            start=False,
                            stop=True,
                        )
                    return ps, nv

                def gen_stats(h, nbq):
                    for qb in range(4 * nbq, 4 * nbq + 4):
                        kmax = qb // 4 + 1 if causal else NB
                        mpart = smalls.tile([P, NB], F32, tag="mpart", name="mpart")
                        nm = 0
                        # NOTE: DVE can read only ONE input from PSUM, so
                        # tiles reduce individually (no tensor_tensor pairing)
                        for kc in range(kmax):
                            ps, nv = stats_mm(h, qb, kc)
                            nc.vector.reduce_max(
                                mpart[:, nm : nm + 1],
                                ps[:, 0:nv],
                                axis=mybir.AxisListType.X,
                            )
                            nm += 1
                            yield
                        nc.vector.tensor_reduce(
                            mcol[h][:, qb : qb + 1],
                            mpart[:, 0:nm],
                            axis=mybir.AxisListType.X,
                            op=mybir.AluOpType.max,
                            negate=True,
                        )
                        nc.vector.tensor_copy(
                            mcol_r[h][:, qb : qb + 1], mcol[h][:, qb : qb + 1]
                        )
                        nc.sync.dma_start(
                            qhT[h][DK : DK + 1, qb * P : (qb + 1) * P],
                            mcol_r[h][:, qb : qb + 1],
                        )
                        yield

                def gen_wo(nbq):
                    for j in range(4):
                        qc = nbq * 4 + j
                        for eb in range(2):
                            psy = ps_proj.tile([P, 512], F32, tag="proj", name="psy")
                            nc.tensor.matmul(
                                psy[:],
                                ct[0:P, qc * P : (qc + 1) * P],
                                wo_sb[0:P, eb * 512 : (eb + 1) * 512],
                                start=True,
                                stop=True,
                            )
                            ysb = ypool.tile([P, 512], BF16, tag="ysb", name="ysb")
                            if eb == 0:
                                nc.scalar.copy(ysb[:], psy[:])
                            else:
                                nc.vector.tensor_copy(ysb[:], psy[:])
                            nc.sync.dma_start(
                                y0[qc * P : (qc + 1) * P, eb * 512 : (eb + 1) * 512],
                                ysb[:],
                            )
                            yield

                def drain(gens):
                    for g in gens:
                        for _ in g:
                            pass

                # ------- main block: both heads' S^T/exp/AV interleaved -------

                def st3_pair(nb, fillers, carry):
                    # anything the current block reads must be fully emitted
                    while fillers and fillers[0][2] <= nb:
                        drain([fillers.popleft()[0]])
                    nkc = 4 * (nb + 1) if causal else QB
                    po = [
                        ps_misc.tile([P, 512], F32, tag="misc", name=f"po{h}")
                        for h in range(NH)
                    ]
                    pss = {}

                    def s_mm(h, kc):
                        ps = ps_st.tile([P, 512], F32, tag="st", name="ps_st")
                        diag = causal and (kc >= 4 * nb)
                        o = kc - 4 * nb if diag else 0
                        qoff = o * P
                        nv = 512 - qoff
                        kslice = slice(kc * P, (kc + 1) * P)
                        qslice = slice(nb * 512 + qoff, (nb + 1) * 512)
                        nc.tensor.matmul(
                            ps[:, 0:nv],
                            khT[h][0 : DK + 1, kslice],
                            qhT[h][0 : DK + 1, qslice],
                            start=True,
                            stop=not diag,
                        )
                        if diag:
                            nc.tensor.matmul(
                                ps[:, 0:P],
                                ident_b[:],
                                mf_sb[:],
                                start=False,
                                stop=True,
                            )
                        pss[(h, kc)] = (ps, qoff, nv)

                    def pexp_av(h, kc):
                        ps, qoff, nv = pss.pop((h, kc))
                        pt = ptp.tile([P, 512], F32R, tag="pt", name="pt")
                        nc.scalar.activation(pt[:, 0:nv], ps[:, 0:nv], EXP)
                        nc.tensor.matmul(
                            po[h][0 : DK + 1, qoff:512],
                            vh[h][:, kc, :],
                            pt[:, 0:nv],
                            start=(kc == 0),
                            stop=(kc == nkc - 1),
                        )

                    n_slots = 4 * nkc
                    n_mand = sum(e[1] for e in fillers)
                    n_carry = sum(e[1] for e in carry)
                    # mandatory fillers front-loaded into the first ~60% of
                    # slots: the next block's st3 is gated on the LAST stats
                    # finalize, so its DVE work must clear well before the
                    # block ends. carry (wo/rscale) paced across everything.
                    ffrac = float(os.environ.get("ATTN_FFRAC", "0.6"))
                    cfrac = float(os.environ.get("ATTN_CFRAC", "1.0"))
                    q_mand = n_mand / max(ffrac * n_slots, 1.0)
                    q_carry = n_carry / max(cfrac * n_slots, 1.0)
                    acc = [0.0, 0.0]

                    def pop1(q):
                        if not q:
                            return False
                        try:
                            next(q[0][0])
                            q[0][1] -= 1
                        except StopIteration:
                            q.popleft()
                            return pop1(q)
                        return True

                    def fill():
                        acc[0] += q_mand
                        acc[1] += q_carry
                        while acc[0] >= 1.0:
                            acc[0] -= 1.0
                            if not pop1(fillers):
                                acc[0] = 0.0
                                break
                        while acc[1] >= 1.0:
                            acc[1] -= 1.0
                            if not pop1(carry):
                                acc[1] = 0.0
                                break

                    s_mm(0, 0)
                    s_mm(1, 0)
                    for kc in range(nkc):
                        if kc + 1 < nkc:
                            s_mm(0, kc + 1)
                        fill()
                        pexp_av(0, kc)
                        fill()
                        if kc + 1 < nkc:
                            s_mm(1, kc + 1)
                        fill()
                        pexp_av(1, kc)
                        fill()
                    # entries for block nb+1 must finish before the next st3;
                    # lookahead entries (nb+2) may flow into the next block
                    while fillers and fillers[0][2] <= nb + 1:
                        if not pop1(fillers):
                            break
                    return po

                def gen_podrain(nb, po):
                    # drain po -> ct + l row. Must run early in block nb+1
                    # (its st3 needs the po PSUM banks back).
                    for h in range(NH):
                        nc.scalar.copy(
                            ct[h * DK : (h + 1) * DK, nb * 512 : (nb + 1) * 512],
                            po[h][0:DK, :],
                        )
                        nc.vector.tensor_copy(
                            qhT[h][96:97, nb * 512 : (nb + 1) * 512],
                            po[h][DK : DK + 1, :],
                        )
                        yield

                def gen_rscale(nb):
                    # r = 1/l; ct *= broadcast(r). Deferrable arbitrarily.
                    for h in range(NH):
                        for j in range(4):
                            qb = nb * 4 + j
                            nc.sync.dma_start(
                                lcol[h][:, qb : qb + 1],
                                qhT[h][96:97, qb * P : (qb + 1) * P],
                            )
                        nc.vector.reciprocal(
                            rcol[h][:, nb * 4 : nb * 4 + 4],
                            lcol[h][:, nb * 4 : nb * 4 + 4],
                        )
                        nc.vector.tensor_copy(
                            rcol_r[h][:, nb * 4 : nb * 4 + 4],
                            rcol[h][:, nb * 4 : nb * 4 + 4],
                        )
                        for j in range(4):
                            qb = nb * 4 + j
                            nc.sync.dma_start(
                                rrow[h : h + 1, qb * P : (qb + 1) * P],
                                rcol_r[h][:, qb : qb + 1],
                            )
                        yield
                    R_ps = ps_stat.tile([P, 512], F32, tag="stat", name="Rps")
                    nc.tensor.matmul(
                        R_ps[:],
                        sel[:, :],
                        rrow[0:2, nb * 512 : (nb + 1) * 512],
                        start=True,
                        stop=True,
                    )
                    nc.vector.tensor_mul(
                        out=ct[:, nb * 512 : (nb + 1) * 512],
                        in0=ct[:, nb * 512 : (nb + 1) * 512],
                        in1=R_ps[:],
                    )
                    yield

                # ------- prologue: block 0 projections + stats -------
                drain([gen_qkproj(1, 0), gen_qkproj(0, 0), gen_vproj(0)])
                drain([gen_stats(0, 0), gen_stats(1, 0)])

                def stats_pieces(nbq):
                    return 4 * (nbq + 2)

                def append_block_gens(mand, x):
                    # proj + stats for block x; FIFO order respects the data
                    # deps (stats reads the K/Q projections)
                    if x < NB:
                        mand.append([gen_qkproj(1, x), 4, x])
                        mand.append([gen_qkproj(0, x), 4, x])
                        mand.append([gen_stats(0, x), stats_pieces(x), x])
                        mand.append([gen_stats(1, x), stats_pieces(x), x])
                        mand.append([gen_vproj(x), 6, x])

                LOOKAHEAD = int(os.environ.get("ATTN_LOOKAHEAD", "3"))
                prev_po = None
                mand = deque()
                carry = deque()
                # prologue: block 0 gens drain fully; lookahead queues behind
                append_block_gens(mand, 0)
                while mand and mand[0][2] <= 0:
                    drain([mand.popleft()[0]])
                for x in range(1, LOOKAHEAD):
                    append_block_gens(mand, x)
                for nb in range(NB):
                    if prev_po is not None:
                        drain([gen_podrain(nb - 1, prev_po)])
                        carry.append([gen_rscale(nb - 1), 3])
                        carry.append([gen_wo(nb - 1), 8])
                    append_block_gens(mand, nb + LOOKAHEAD)
                    prev_po = st3_pair(nb, mand, carry)
                carry.append([gen_podrain(NB - 1, prev_po), 2])
                carry.append([gen_rscale(NB - 1), 3])
                carry.append([gen_wo(NB - 1), 8])
                drain([g for g, *_ in carry])

    _split_waits(nc)
    return nc


_cache = {}


def _get_nc(causal: bool):
    if causal not in _cache:
        nc = bass.Bass(trn_type="TRN2")
        build(nc, causal=causal)
        _cache[causal] = nc
    return _cache[causal]


def _host_masks():
    p = np.arange(P)[:, None]
    j = np.arange(P)[None, :]
    # S^T diag tile [kc, q]: nonzero only in the first 128 q-cols: p > j
    maskf = np.where(p > j, NEG, 0.0).astype(ml_dtypes.bfloat16)
    # stats diag tile [q, kc]: nonzero only in the last 128 kc-cols: j > p
    maskb = np.where(j > p, NEG, 0.0).astype(ml_dtypes.bfloat16)
    return maskf, maskb


def make_in_maps(np_inputs):
    Q = np.asarray(np_inputs["Q"], dtype=np.float32)
    K = np.asarray(np_inputs["K"], dtype=np.float32)
    V = np.asarray(np_inputs["V"], dtype=np.float32)
    W_Q = np.asarray(np_inputs["W_Q"], dtype=np.float32)
    W_K = np.asarray(np_inputs["W_K"], dtype=np.float32)
    W_V = np.asarray(np_inputs["W_V"], dtype=np.float32)
    W_O = np.asarray(np_inputs["W_O"], dtype=np.float32)

    qTh = np.ascontiguousarray(Q.T)
    kTh = np.ascontiguousarray(K.T)
    vTh = np.ascontiguousarray(V.T)
    maskf, maskb = _host_masks()
    ones_row = np.ones((1, S), dtype=np.float32)
    sel_rows = np.zeros((2, P), dtype=np.float32)
    sel_rows[0, 0:DK] = 1.0
    sel_rows[1, DK:P] = 1.0

    def wtile(w2):
        # [D, M] -> [P, DC*M]: row p holds the 128-row contraction chunks
        M = w2.shape[1]
        return np.ascontiguousarray(
            w2.reshape(D // P, P, M).transpose(1, 0, 2).reshape(P, (D // P) * M)
        )

    scale = np.float32(1.0 / np.sqrt(DK))
    in_maps = []
    for c in range(NCORES):
        h0, h1 = 2 * c, 2 * c + 1
        wq2 = wtile(
            np.concatenate([W_Q[h0] * scale, W_Q[h1] * scale], axis=1).astype(
                np.float32
            )
        )
        wk2 = wtile(np.concatenate([W_K[h0], W_K[h1]], axis=1))
        wv2 = wtile(np.concatenate([W_V[h0], W_V[h1]], axis=1))
        wo2 = np.ascontiguousarray(W_O[P * c : P * (c + 1), :])
        in_maps.append(
            {
                "qT": qTh,
                "kT": kTh,
                "vT": vTh,
                "wq": wq2,
                "wk": wk2,
                "wv": wv2,
                "wo": wo2,
                "maskf": maskf,
                "maskb": maskb,
                "onesr": ones_row,
                "selr": sel_rows,
            }
        )
    return in_maps


LAST_EXEC_NS = None
LAST_RES = None


def kernel(Q, K, V, W_Q, W_K, W_V, W_O, mask):
    global LAST_EXEC_NS, LAST_RES
    causal = bool(np.asarray(mask).item())
    nc = _get_nc(causal)
    in_maps = make_in_maps(
        dict(Q=Q, K=K, V=V, W_Q=W_Q, W_K=W_K, W_V=W_V, W_O=W_O)
    )

    trace = bool(int(os.environ.get("ATTN_TRACE", "0")))
    res = run_bass_kernel_spmd(
        nc, in_maps, core_ids=list(range(NCORES)), trace=trace
    )
    LAST_EXEC_NS = res.exec_time_ns
    LAST_RES = res

    out = np.zeros((S, D), dtype=np.float32)
    for c in range(NCORES):
        out += np.asarray(res.results[c]["y0"], dtype=np.float32)
    return out
